# revision 18
# baseline (speedup 1.0000x reference)
"""Multi-head causal attention with RoPE on 8 TRN2 NeuronCores.

Sharding: 8 cores = 2 batches x 4 head-groups (4 heads each).
Per-core Bass kernel computes the group-partial output transposed;
host sums group partials and transposes back.

v2: fully fused single pass (no DRAM staging of Q/K/V), bf16 matmul
operands with f32 PSUM accumulation, softmax denominator accumulated
on DVE (bf16) with one ones-matmul per (u, head) chain, DMA issue
spread across idle engine queues.
"""

import numpy as np

import concourse.bass as bass  # noqa: F401
import concourse.tile as tile
from concourse import bacc, mybir

B, S, D, H, HD = 2, 2048, 2048, 16, 128
NCORES = 8
G = 4            # head groups
GH = 4           # heads per group
GD = GH * HD     # 512 dims per group
P = 128
SQ_U = S // 512  # 4 query slices
SK_T = S // P    # 16 key tiles

_f32 = mybir.dt.float32
_f32r = mybir.dt.float32r
_bf16 = mybir.dt.bfloat16
_np_bf16 = mybir.dt.np(_bf16)

_cache = {}


def _build(causal: bool, reps: int = 1, pq_bufs: int = 2, pv_bufs: int = 2,
           s_bufs: int = 3, a_bufs: int = 1, depth: int = 2, app_bufs: int = 6,
           den_bufs: int = 2, so_bufs: int = 2, rope_bufs: int = 2,
           w_stride: int = 2, phases: str = "paw"):
    if not causal:
        return _build_generic(reps)
    nc = bacc.Bacc("TRN2", target_bir_lowering=False, debug=False)
    xT = nc.dram_tensor("xT", [D, S], _bf16, kind="ExternalInput").ap()
    wq = nc.dram_tensor("wq", [D, GD], _bf16, kind="ExternalInput").ap()
    wk = nc.dram_tensor("wk", [D, GD], _bf16, kind="ExternalInput").ap()
    wv = nc.dram_tensor("wv", [D, GD], _bf16, kind="ExternalInput").ap()
    wo = nc.dram_tensor("wo", [GD, D], _bf16, kind="ExternalInput").ap()
    cs = nc.dram_tensor("cs", [P, S], _bf16, kind="ExternalInput").ap()
    ss = nc.dram_tensor("ss", [P, S], _bf16, kind="ExternalInput").ap()
    ones = nc.dram_tensor("ones", [P, P], _bf16, kind="ExternalInput").ap()
    # triangle blocks of the causal mask: one [P, P] block per key tile
    maskd = nc.dram_tensor("maskd", [P, SK_T * P], _bf16, kind="ExternalInput").ap()
    outT = nc.dram_tensor("outT", [D, S], _bf16, kind="ExternalOutput").ap()

    with tile.TileContext(nc) as tc:
        with (
            tc.tile_pool(name="persist", bufs=1) as persist,
            tc.tile_pool(name="kvp", bufs=2) as kvp,
            tc.tile_pool(name="xp", bufs=2) as xp,
            tc.tile_pool(name="qtp", bufs=2) as qtp,
            tc.tile_pool(name="aotp", bufs=2) as aotp,
            tc.tile_pool(name="ropep", bufs=rope_bufs) as ropep,
            tc.tile_pool(name="app", bufs=app_bufs) as app,
            tc.tile_pool(name="denp", bufs=den_bufs) as denp,
            tc.tile_pool(name="recp", bufs=1) as recp,
            tc.tile_pool(name="sop", bufs=so_bufs) as sop,
            tc.tile_pool(name="ps_q", bufs=pq_bufs, space="PSUM") as ps_q,
            tc.tile_pool(name="ps_v", bufs=pv_bufs, space="PSUM") as ps_v,
            tc.tile_pool(name="ps_s", bufs=s_bufs, space="PSUM") as ps_s,
            tc.tile_pool(name="ps_a", bufs=a_bufs, space="PSUM") as ps_a,
        ):
          for _rep in range(reps):
            # ---- per-body SBUF tensors (bufs=1 rotation orders reps)
            wq_s = persist.tile([P, SK_T * GD], _bf16, tag="wq")
            wk_s = persist.tile([P, SK_T * GD], _bf16, tag="wk")
            wv_s = persist.tile([P, SK_T * GD], _bf16, tag="wv")
            wo_s = persist.tile([P, GH * S], _bf16, tag="wo")
            cs_s = persist.tile([P, S], _bf16, tag="cs")
            ss_s = persist.tile([P, S], _bf16, tag="ss")
            md_s = persist.tile([P, SK_T * P], _bf16, tag="md")
            ones_s = persist.tile([P, P], _bf16, tag="ones")
            kt = kvp.tile([P, GH * S], _bf16, tag="kt")    # K^T per head
            vn = kvp.tile([P, SK_T * GD], _bf16, tag="vn")  # V natural

            # ---- initial loads, spread across engine queues
            # SP: wq tiles (needed first), then small constants
            nc.sync.dma_start(ones_s[:], ones[:])
            nc.sync.dma_start(cs_s[:], cs[:])
            nc.sync.dma_start(ss_s[:], ss[:])
            for t in range(SK_T):
                nc.sync.dma_start(wq_s[:, t * GD:(t + 1) * GD], wq[t * P:(t + 1) * P, :])
            # ACT queue: wk, wv then wo (needed later)
            for t in range(SK_T):
                nc.scalar.dma_start(wk_s[:, t * GD:(t + 1) * GD], wk[t * P:(t + 1) * P, :])
                nc.scalar.dma_start(wv_s[:, t * GD:(t + 1) * GD], wv[t * P:(t + 1) * P, :])
            for dt in range(GH):
                nc.scalar.dma_start(wo_s[:, dt * S:(dt + 1) * S], wo[dt * P:(dt + 1) * P, :])

            def _load_xu(u):
                xu = xp.tile([P, SK_T * 512], _bf16, tag="xu")
                for t in range(SK_T):
                    nc.gpsimd.dma_start(
                        xu[:, t * 512:(t + 1) * 512],
                        xT[t * P:(t + 1) * P, u * 512:(u + 1) * 512])
                return xu

            xus = [None] * SQ_U
            xus[0] = _load_xu(0)
            # mask triangle blocks (Pool queue, needed from A(0))
            nc.gpsimd.dma_start(md_s[:], maskd[:])

            aots = [None] * SQ_U

            def _phase_p(u):
                if xus[u] is None:
                    xus[u] = _load_xu(u)
                xu = xus[u]
                qt_u = qtp.tile([P, GH * 512], _bf16, tag="qt")
                # Q^T and K^T (transposed) with fused RoPE
                for dt in range(GH):
                    for (w_s, dst, doff) in (
                            (wq_s, qt_u, dt * 512),
                            (wk_s, kt, dt * S + u * 512)):
                        pq = ps_q.tile([P, 512], _f32, tag="pq")
                        for t in range(SK_T):
                            nc.tensor.matmul(
                                pq[:],
                                w_s[:, t * GD + dt * P: t * GD + dt * P + P],
                                xu[:, t * 512:(t + 1) * 512],
                                start=(t == 0), stop=(t == SK_T - 1))
                        su = slice(u * 512, (u + 1) * 512)
                        t1 = ropep.tile([P, 512], _bf16, tag="t1")
                        t2 = ropep.tile([P, 512], _bf16, tag="t2")
                        nc.vector.tensor_mul(t1[:], pq[:], cs_s[:, su])
                        nc.vector.tensor_mul(t2[0:64, :], pq[64:P, :], ss_s[0:64, su])
                        nc.vector.tensor_mul(t2[64:P, :], pq[0:64, :], ss_s[64:P, su])
                        nc.vector.tensor_add(dst[:, doff:doff + 512], t1[:], t2[:])
                # V (natural layout)
                for st in range(4):
                    g = 4 * u + st
                    pv = ps_v.tile([P, GD], _f32, tag="pv")
                    for t in range(SK_T):
                        nc.tensor.matmul(
                            pv[:],
                            xu[:, t * 512 + st * P: t * 512 + st * P + P],
                            wv_s[:, t * GD:(t + 1) * GD],
                            start=(t == 0), stop=(t == SK_T - 1))
                    nc.scalar.copy(vn[:, g * GD:(g + 1) * GD], pv[:])
                return qt_u

            def _w_group(u, ot):
                # one output-projection group: po2 = sum_dt wo[dt]^T aot[dt]
                aot = aots[u]
                po2 = ps_v.tile([P, 512], _f32, tag="pv")
                for dt in range(GH):
                    nc.tensor.matmul(
                        po2[:],
                        wo_s[:, dt * S + ot * P: dt * S + (ot + 1) * P],
                        aot[:, dt * 512:(dt + 1) * 512],
                        start=(dt == 0), stop=(dt == GH - 1))
                so = sop.tile([P, 512], _bf16, tag="so")
                nc.vector.tensor_scalar_add(so[:], po2[:], 0.0)
                nc.gpsimd.dma_start(
                    outT[ot * P:(ot + 1) * P, u * 512:(u + 1) * 512], so[:])

            def _phase_a(u, qt_u, w_u):
                # prefetch next x slice
                if u + 1 < SQ_U:
                    xus[u + 1] = _load_xu(u + 1)
                aot = aotp.tile([P, GH * 512], _bf16, tag="aot")
                n_sk = 4 * (u + 1)
                w_pend = list(range(SK_T)) if w_u is not None else []
                slot = [0]

                def _w_tick():
                    # interleave one pending W(u-1) group every w_stride slots
                    slot[0] += 1
                    if w_pend and slot[0] % w_stride == 0:
                        _w_group(w_u, w_pend.pop(0))

                for h in range(GH):
                    qu = qt_u[:, h * 512:(h + 1) * 512]
                    psa = ps_a.tile([P, 512], _f32, tag="a")
                    dacc = denp.tile([P, 512], _bf16, tag="dacc")
                    pts = [None] * n_sk
                    rngs = [None] * n_sk

                    def _consume(t):
                        rng = rngs[t]
                        nc.tensor.matmul(psa[:, rng], vn[:, t * GD + h * P: t * GD + (h + 1) * P],
                                         pts[t][:, rng],
                                         start=(t == 0), stop=(t == n_sk - 1))
                        if t == 0:
                            nc.vector.tensor_scalar_add(dacc[:], pts[0][:], 0.0)
                        else:
                            nc.vector.tensor_add(
                                dacc[:, rng], dacc[:, rng], pts[t][:, rng])

                    for t in range(n_sk):
                        # causal trim: diagonal tiles only need q >= (t%4)*P
                        s0 = 0 if t < 4 * u else (t - 4 * u) * P
                        rng = slice(s0, 512)
                        rngs[t] = rng
                        pss = ps_s.tile([P, 512], _f32, tag="s")
                        nc.tensor.matmul(pss[:, rng],
                                         kt[:, h * S + t * P: h * S + (t + 1) * P],
                                         qu[:, rng], start=True, stop=True)
                        if t >= 4 * u:
                            # mask only the leading [P, P] triangle block
                            nc.vector.tensor_add(
                                pss[:, s0:s0 + P], pss[:, s0:s0 + P],
                                md_s[:, t * P:(t + 1) * P])
                        pt = app.tile([P, 512], _bf16, tag="p")
                        nc.scalar.activation(pt[:, rng], pss[:, rng],
                                             mybir.ActivationFunctionType.Exp)
                        pts[t] = pt
                        if t >= depth:
                            _consume(t - depth)
                        _w_tick()
                    for t in range(max(0, n_sk - depth), n_sk):
                        _consume(t)
                        _w_tick()
                    psd = ps_q.tile([P, 512], _f32, tag="pq")
                    nc.tensor.matmul(psd[:], ones_s[:], dacc[:],
                                     start=True, stop=True)
                    rec = recp.tile([P, 512], _f32, tag="rec")
                    nc.vector.reciprocal(rec[:], psd[:])
                    nc.vector.tensor_mul(
                        aot[:, h * 512:(h + 1) * 512], psa[:], rec[:])
                # drain any leftover W groups
                for ot in w_pend:
                    _w_group(w_u, ot)
                return aot

            # order: P0 A0 P1 [W0|A1] P2 [W1|A2] P3 [W2|A3] W3
            do_a, do_w = "a" in phases, "w" in phases
            qt0 = _phase_p(0)
            if do_a:
                aots[0] = _phase_a(0, qt0, None)
            for u in range(1, SQ_U):
                qt_u = _phase_p(u)
                if do_a:
                    aots[u] = _phase_a(u, qt_u, u - 1 if do_w else None)
            if do_w:
                for ot in range(SK_T):
                    _w_group(SQ_U - 1, ot)
    nc.compile()
    return nc


def _build_generic(reps: int = 1, phases: str = "paw",
                   xu_bufs: int = 3, s_bufs: int = 3, o_bufs: int = 1, depth: int = 3,
                   app_bufs: int = 6, aq_bufs: int = 2, pt_bufs: int = 3,
                   pq_bufs: int = 5, pv_bufs: int = 3, po_bufs: int = 4):
    """v1 path for arbitrary (non-causal) masks; f32r, DRAM staging."""
    nc = bacc.Bacc("TRN2", target_bir_lowering=False, debug=False)
    xT = nc.dram_tensor("xT", [D, S], _f32r, kind="ExternalInput").ap()
    wq = nc.dram_tensor("wq", [D, GD], _f32r, kind="ExternalInput").ap()
    wk = nc.dram_tensor("wk", [D, GD], _f32r, kind="ExternalInput").ap()
    wv = nc.dram_tensor("wv", [D, GD], _f32r, kind="ExternalInput").ap()
    wo = nc.dram_tensor("wo", [GD, D], _f32r, kind="ExternalInput").ap()
    cs = nc.dram_tensor("cs", [P, S], _f32, kind="ExternalInput").ap()
    ss = nc.dram_tensor("ss", [P, S], _f32, kind="ExternalInput").ap()
    ones = nc.dram_tensor("ones", [P, P], _f32r, kind="ExternalInput").ap()
    maskf = nc.dram_tensor("maskf", [S, S], _f32, kind="ExternalInput").ap()
    outT = nc.dram_tensor("outT", [D, S], _f32, kind="ExternalOutput").ap()

    with tile.TileContext(nc) as tc:
      for _rep in range(reps):
        with (
            tc.tile_pool(name="persist", bufs=1) as persist,
            tc.tile_pool(name="dram", bufs=1, space="DRAM") as dpool,
        ):
            qtd = dpool.tile([P, GH * S], _f32r, tag="qtd")   # Q^T per head [hd, S]
            ktd = dpool.tile([P, GH * S], _f32r, tag="ktd")
            vd = dpool.tile([P, SK_T * GD], _f32r, tag="vd")  # V natural
            ones_s = persist.tile([P, P], _f32r, tag="ones")
            nc.sync.dma_start(ones_s[:], ones[:])

            # ---- Phase P: Q^T/K^T/V projections + RoPE -> DRAM scratch
            with (
                tc.tile_pool(name="pw", bufs=1) as pw,
                tc.tile_pool(name="px", bufs=xu_bufs) as px,
                tc.tile_pool(name="pt", bufs=pt_bufs) as ptp,
                tc.tile_pool(name="po", bufs=po_bufs) as po,
                tc.tile_pool(name="ps1", bufs=pq_bufs, space="PSUM") as ps1,
                tc.tile_pool(name="ps2", bufs=pv_bufs, space="PSUM") as ps2,
            ):
                wq_s = pw.tile([P, SK_T * GD], _f32r, tag="wq")
                wk_s = pw.tile([P, SK_T * GD], _f32r, tag="wk")
                wv_s = pw.tile([P, SK_T * GD], _f32r, tag="wv")
                cs_s = pw.tile([P, S], _f32, tag="cs")
                ss_s = pw.tile([P, S], _f32, tag="ss")

                def _load_xu(u):
                    xh = []
                    for half in range(2):
                        xu = px.tile([P, 8 * 512], _f32r, tag="xu")
                        for j in range(8):
                            t = half * 8 + j
                            nc.sync.dma_start(
                                xu[:, j * 512:(j + 1) * 512],
                                xT[t * P:(t + 1) * P, u * 512:(u + 1) * 512])
                        xh.append(xu)
                    return xh

                nc.sync.dma_start(cs_s[:], cs[:])
                nc.sync.dma_start(ss_s[:], ss[:])
                xh0 = _load_xu(0)
                for t in range(SK_T):
                    nc.sync.dma_start(wq_s[:, t * GD:(t + 1) * GD], wq[t * P:(t + 1) * P, :])
                for t in range(SK_T):
                    nc.sync.dma_start(wk_s[:, t * GD:(t + 1) * GD], wk[t * P:(t + 1) * P, :])
                    nc.sync.dma_start(wv_s[:, t * GD:(t + 1) * GD], wv[t * P:(t + 1) * P, :])
                for u in range(SQ_U):
                    xh = xh0 if u == 0 else _load_xu(u)
                    su = slice(u * 512, (u + 1) * 512)
                    for (w_s, dst) in ((wq_s, qtd), (wk_s, ktd)):
                        for dt in range(GH):
                            pq = ps1.tile([P, 512], _f32, tag="pq")
                            for t in range(SK_T):
                                nc.tensor.matmul(
                                    pq[:],
                                    w_s[:, t * GD + dt * P: t * GD + dt * P + P],
                                    xh[t // 8][:, (t % 8) * 512:(t % 8 + 1) * 512],
                                    start=(t == 0), stop=(t == SK_T - 1))
                            t1 = ptp.tile([P, 512], _f32, tag="t1")
                            t2 = ptp.tile([P, 512], _f32, tag="t2")
                            nc.vector.tensor_mul(t1[:], pq[:], cs_s[:, su])
                            nc.vector.tensor_mul(t2[0:64, :], pq[64:P, :], ss_s[0:64, su])
                            nc.vector.tensor_mul(t2[64:P, :], pq[0:64, :], ss_s[64:P, su])
                            ro = po.tile([P, 512], _f32r, tag="ro")
                            nc.vector.tensor_add(ro[:], t1[:], t2[:])
                            nc.sync.dma_start(
                                dst[:, dt * S + u * 512: dt * S + (u + 1) * 512], ro[:])
                    for st in range(4):
                        g = 4 * u + st
                        pv = ps2.tile([P, GD], _f32, tag="pv")
                        for t in range(SK_T):
                            nc.tensor.matmul(
                                pv[:],
                                xh[t // 8][:, (t % 8) * 512 + st * P: (t % 8) * 512 + st * P + P],
                                wv_s[:, t * GD:(t + 1) * GD],
                                start=(t == 0), stop=(t == SK_T - 1))
                        vo = po.tile([P, GD], _f32r, tag="vo")
                        nc.scalar.copy(vo[:], pv[:])
                        nc.sync.dma_start(vd[:, g * GD:(g + 1) * GD], vo[:])

            # ---- Phases A (attention) + W (output projection)
            with (
                tc.tile_pool(name="amask", bufs=1) as amask_p,
                tc.tile_pool(name="akv", bufs=1) as akv,
                tc.tile_pool(name="aq", bufs=aq_bufs) as aq,
                tc.tile_pool(name="app", bufs=app_bufs) as app,
                tc.tile_pool(name="ar", bufs=2) as ar,
                tc.tile_pool(name="aw", bufs=1) as aw,
                tc.tile_pool(name="wst", bufs=3) as wst,
                tc.tile_pool(name="ps3", bufs=s_bufs, space="PSUM") as ps3,
                tc.tile_pool(name="ps3b", bufs=2, space="PSUM") as ps3b,
                tc.tile_pool(name="ps4", bufs=o_bufs, space="PSUM") as ps4,
            ):
                wo_s = aw.tile([P, GH * S], _f32r, tag="wo")
                aot = aw.tile([P, GH * S], _f32r, tag="aot")
                for dt in range(GH):
                    nc.sync.dma_start(wo_s[:, dt * S:(dt + 1) * S], wo[dt * P:(dt + 1) * P, :])
                kt_all = akv.tile([P, GH * S], _f32r, tag="kt")
                v_all = akv.tile([P, GH * S], _f32r, tag="vh")
                for u in range(SQ_U):
                    for lu in (range(SQ_U) if u == 0 else []):
                        for h in range(GH):
                            nc.sync.dma_start(
                                kt_all[:, h * S + lu * 512: h * S + (lu + 1) * 512],
                                ktd[:, h * S + lu * 512: h * S + (lu + 1) * 512])
                            for st in range(4):
                                t = 4 * lu + st
                                nc.sync.dma_start(
                                    v_all[:, h * S + t * P: h * S + (t + 1) * P],
                                    vd[:, t * GD + h * P: t * GD + (h + 1) * P])
                    for h in range(GH):
                        kt_h = kt_all[:, h * S: (h + 1) * S]
                        v_h = v_all[:, h * S: (h + 1) * S]
                        qu = aq.tile([P, 512], _f32r, tag="qu")
                        nc.sync.dma_start(
                            qu[:], qtd[:, h * S + u * 512: h * S + (u + 1) * 512])
                        mu = amask_p.tile([P, SK_T * 512], _f32, tag="mu")
                        for t in range(SK_T):
                            nc.sync.dma_start(
                                mu[:, t * 512:(t + 1) * 512],
                                maskf[t * P:(t + 1) * P, u * 512:(u + 1) * 512])
                        n_sk = SK_T
                        psd = ps3b.tile([P, 512], _f32, tag="d")
                        psa = ps3b.tile([P, 512], _f32, tag="a")
                        pts = [None] * n_sk

                        def _consume(t):
                            nc.tensor.matmul(psd[:], ones_s[:], pts[t][:],
                                             start=(t == 0), stop=(t == n_sk - 1))
                            nc.tensor.matmul(psa[:], v_h[:, t * P:(t + 1) * P], pts[t][:],
                                             start=(t == 0), stop=(t == n_sk - 1))

                        for t in range(n_sk):
                            pss = ps3.tile([P, 512], _f32, tag="s")
                            nc.tensor.matmul(pss[:], kt_h[:, t * P:(t + 1) * P], qu[:],
                                             start=True, stop=True)
                            nc.vector.tensor_add(
                                pss[:], pss[:], mu[:, t * 512:(t + 1) * 512])
                            pt = app.tile([P, 512], _f32r, tag="p")
                            nc.scalar.activation(pt[:], pss[:],
                                                 mybir.ActivationFunctionType.Exp)
                            pts[t] = pt
                            if t >= depth:
                                _consume(t - depth)
                        for t in range(max(0, n_sk - depth), n_sk):
                            _consume(t)
                        rec = ar.tile([P, 512], _f32, tag="rec")
                        nc.vector.reciprocal(rec[:], psd[:])
                        nc.vector.tensor_mul(
                            aot[:, h * S + u * 512: h * S + (u + 1) * 512],
                            psa[:], rec[:])
                    for ot in range(SK_T):
                        po2 = ps4.tile([P, 512], _f32, tag="o")
                        for dt in range(GH):
                            nc.tensor.matmul(
                                po2[:],
                                wo_s[:, dt * S + ot * P: dt * S + (ot + 1) * P],
                                aot[:, dt * S + u * 512: dt * S + (u + 1) * 512],
                                start=(dt == 0), stop=(dt == GH - 1))
                        so = wst.tile([P, 512], _f32, tag="so")
                        nc.scalar.copy(so[:], po2[:])
                        nc.sync.dma_start(
                            outT[ot * P:(ot + 1) * P, u * 512:(u + 1) * 512], so[:])
    nc.compile()
    return nc


class _Runner:
    """Persistent PJRT executable for one compiled Bass module (SPMD over 8 cores)."""

    def __init__(self, nc, n_cores):
        import jax
        from jax.sharding import Mesh, PartitionSpec
        from jax.experimental.shard_map import shard_map
        from concourse.bass2jax import (
            _bass_exec_p, install_neuronx_cc_hook, partition_id_tensor)

        install_neuronx_cc_hook()
        self.jax = jax
        self.n_cores = n_cores
        partition_name = nc.partition_id_tensor.name if nc.partition_id_tensor else None
        in_names, out_names, out_avals = [], [], []
        for alloc in nc.m.functions[0].allocations:
            if not isinstance(alloc, mybir.MemoryLocationSet):
                continue
            name = alloc.memorylocations[0].name
            if alloc.kind == "ExternalInput":
                if name != partition_name:
                    in_names.append(name)
            elif alloc.kind == "ExternalOutput":
                out_names.append(name)
                out_avals.append(jax.core.ShapedArray(
                    tuple(alloc.tensor_shape), mybir.dt.np(alloc.dtype)))
        self.in_names, self.out_names, self.out_avals = in_names, out_names, out_avals
        n_params, n_outs = len(in_names), len(out_avals)
        all_in = list(in_names) + list(out_names)
        if partition_name is not None:
            all_in.append(partition_name)

        def _body(*args):
            operands = list(args)
            if partition_name is not None:
                operands.append(partition_id_tensor())
            return tuple(_bass_exec_p.bind(
                *operands,
                out_avals=tuple(out_avals), in_names=tuple(all_in),
                out_names=tuple(out_names), lowering_input_output_aliases=(),
                sim_require_finite=True, sim_require_nnan=True, nc=nc))

        devices = jax.devices()[:n_cores]
        mesh = Mesh(np.asarray(devices), ("core",))
        self.sharding = jax.sharding.NamedSharding(mesh, PartitionSpec("core"))
        self.fn = jax.jit(
            shard_map(_body, mesh=mesh,
                      in_specs=(PartitionSpec("core"),) * (n_params + n_outs),
                      out_specs=(PartitionSpec("core"),) * n_outs,
                      check_rep=False),
            keep_unused=True)
        self._dev_args = None

    def put_inputs(self, in_maps):
        jax = self.jax
        concat_in = [
            np.concatenate([np.asarray(in_maps[c][n]) for c in range(self.n_cores)], axis=0)
            for n in self.in_names]
        concat_zeros = [
            np.zeros((self.n_cores * a.shape[0], *a.shape[1:]), a.dtype)
            for a in self.out_avals]
        self._dev_args = [
            jax.device_put(v, self.sharding) for v in concat_in + concat_zeros]
        for a in self._dev_args:
            a.block_until_ready()

    def execute(self):
        return self.fn(*self._dev_args)

    def run(self, in_maps):
        last_err = None
        for attempt in range(3):
            try:
                self.put_inputs(in_maps)
                outs = self.execute()
                self.jax.block_until_ready(outs)
                return [
                    {n: np.asarray(outs[i]).reshape(
                        self.n_cores, *self.out_avals[i].shape)[c]
                     for i, n in enumerate(self.out_names)}
                    for c in range(self.n_cores)]
            except Exception as e:  # transient NRT faults: retry
                last_err = e
                import time
                time.sleep(2.0 * (attempt + 1))
        raise last_err


def _get_runner(causal: bool):
    if causal not in _cache:
        _cache[causal] = _Runner(_build(causal), NCORES)
    return _cache[causal]


def _host_prep(x, mask, Wq, Wk, Wv, Wo, causal):
    scale = np.float32(1.0) / np.sqrt(np.float32(HD))
    perm = np.concatenate(
        [np.concatenate([np.arange(0, HD, 2), np.arange(1, HD, 2)]) + HD * hh
         for hh in range(GH)])
    inv = (np.float32(1.0) / np.power(
        np.float32(10000.0),
        np.arange(0, HD, 2).astype(np.float32) / np.float32(HD))).astype(np.float32)
    ang = np.arange(S, dtype=np.float32)[:, None] * inv[None, :]
    cos_t = np.cos(ang).T.astype(np.float32)
    sin_t = np.sin(ang).T.astype(np.float32)
    cs_host = np.ascontiguousarray(np.concatenate([cos_t, cos_t], axis=0))
    ss_host = np.ascontiguousarray(np.concatenate([-sin_t, sin_t], axis=0))
    maskT = np.ascontiguousarray(mask.T)
    if causal:
        dt_w = _np_bf16
        # per key-tile triangle block: mask[q, k].T sliced at the diagonal
        md = np.empty((P, SK_T * P), np.float32)
        for t in range(SK_T):
            q0 = (t // 4) * 512 + (t % 4) * P
            md[:, t * P:(t + 1) * P] = maskT[t * P:(t + 1) * P, q0:q0 + P]
    else:
        dt_w = np.float32
    ones_host = np.ones((P, P), dt_w)
    xTs = [np.ascontiguousarray(x[b].T).astype(dt_w) for b in range(B)]
    in_maps = []
    for c in range(NCORES):
        b, g = c // G, c % G
        rows = slice(g * GD, (g + 1) * GD)
        m = {
            "xT": xTs[b],
            "wq": np.ascontiguousarray(Wq[rows].T[:, perm] * scale).astype(dt_w),
            "wk": np.ascontiguousarray(Wk[rows].T[:, perm]).astype(dt_w),
            "wv": np.ascontiguousarray(Wv[rows].T).astype(dt_w),
            "wo": np.ascontiguousarray(Wo[:, rows].T).astype(dt_w),
            "cs": cs_host.astype(dt_w) if causal else cs_host,
            "ss": ss_host.astype(dt_w) if causal else ss_host,
            "ones": ones_host,
        }
        if causal:
            m["maskd"] = md.astype(_np_bf16)
        else:
            m["maskf"] = maskT
        in_maps.append(m)
    return in_maps


def kernel(x, mask, Wq, Wk, Wv, Wo):
    x = np.asarray(x, dtype=np.float32)
    mask = np.asarray(mask, dtype=np.float32)
    Wq = np.asarray(Wq, dtype=np.float32)
    Wk = np.asarray(Wk, dtype=np.float32)
    Wv = np.asarray(Wv, dtype=np.float32)
    Wo = np.asarray(Wo, dtype=np.float32)
    expected_mask = np.triu(np.full((S, S), -1e9, dtype=np.float32), k=1)
    causal = bool(np.array_equal(mask, expected_mask))
    runner = _get_runner(causal)
    in_maps = _host_prep(x, mask, Wq, Wk, Wv, Wo, causal)
    results = runner.run(in_maps)
    out = np.empty((B, S, D), np.float32)
    for b in range(B):
        acc = results[b * G]["outT"].astype(np.float32)
        for g in range(1, G):
            acc += results[b * G + g]["outT"].astype(np.float32)
        out[b] = acc.T
    return out


# revision 19
# speedup vs baseline: 1.0667x; 1.0667x over previous
"""Multi-head causal attention with RoPE on 8 TRN2 NeuronCores.

Sharding: 8 cores = 2 batches x 4 head-groups (4 heads each).
Per-core Bass kernel computes the group-partial output transposed;
host sums group partials and transposes back.

v2: fully fused single pass (no DRAM staging of Q/K/V), bf16 matmul
operands with f32 PSUM accumulation, softmax denominator accumulated
on DVE (bf16) with one ones-matmul per (u, head) chain, DMA issue
spread across idle engine queues.
"""

import numpy as np

import concourse.bass as bass  # noqa: F401
import concourse.tile as tile
from concourse import bacc, mybir

B, S, D, H, HD = 2, 2048, 2048, 16, 128
NCORES = 8
G = 4            # head groups
GH = 4           # heads per group
GD = GH * HD     # 512 dims per group
P = 128
SQ_U = S // 512  # 4 query slices
SK_T = S // P    # 16 key tiles

_f32 = mybir.dt.float32
_f32r = mybir.dt.float32r
_bf16 = mybir.dt.bfloat16
_np_bf16 = mybir.dt.np(_bf16)

_cache = {}


def _build(causal: bool, reps: int = 1, pq_bufs: int = 2, pv_bufs: int = 2,
           s_bufs: int = 3, a_bufs: int = 1, depth: int = 2, app_bufs: int = 6,
           den_bufs: int = 2, so_bufs: int = 2, rope_bufs: int = 2,
           w_stride: int = 2, phases: str = "paw"):
    if not causal:
        return _build_generic(reps)
    nc = bacc.Bacc("TRN2", target_bir_lowering=False, debug=False)
    xT = nc.dram_tensor("xT", [D, S], _bf16, kind="ExternalInput").ap()
    wq = nc.dram_tensor("wq", [D, GD], _bf16, kind="ExternalInput").ap()
    wk = nc.dram_tensor("wk", [D, GD], _bf16, kind="ExternalInput").ap()
    wv = nc.dram_tensor("wv", [D, GD], _bf16, kind="ExternalInput").ap()
    wo = nc.dram_tensor("wo", [GD, D], _bf16, kind="ExternalInput").ap()
    cs = nc.dram_tensor("cs", [P, S], _bf16, kind="ExternalInput").ap()
    ss = nc.dram_tensor("ss", [P, S], _bf16, kind="ExternalInput").ap()
    ones = nc.dram_tensor("ones", [P, P], _bf16, kind="ExternalInput").ap()
    # triangle blocks of the causal mask: one [P, P] block per key tile
    maskd = nc.dram_tensor("maskd", [P, SK_T * P], _bf16, kind="ExternalInput").ap()
    outT = nc.dram_tensor("outT", [D, S], _bf16, kind="ExternalOutput").ap()

    with tile.TileContext(nc) as tc:
        with (
            tc.tile_pool(name="persist", bufs=1) as persist,
            tc.tile_pool(name="kvp", bufs=2) as kvp,
            tc.tile_pool(name="xp", bufs=2) as xp,
            tc.tile_pool(name="qtp", bufs=2) as qtp,
            tc.tile_pool(name="aotp", bufs=2) as aotp,
            tc.tile_pool(name="ropep", bufs=rope_bufs) as ropep,
            tc.tile_pool(name="app", bufs=app_bufs) as app,
            tc.tile_pool(name="denp", bufs=den_bufs) as denp,
            tc.tile_pool(name="recp", bufs=1) as recp,
            tc.tile_pool(name="sop", bufs=so_bufs) as sop,
            tc.tile_pool(name="ps_q", bufs=pq_bufs, space="PSUM") as ps_q,
            tc.tile_pool(name="ps_v", bufs=pv_bufs, space="PSUM") as ps_v,
            tc.tile_pool(name="ps_s", bufs=s_bufs, space="PSUM") as ps_s,
            tc.tile_pool(name="ps_a", bufs=a_bufs, space="PSUM") as ps_a,
        ):
          for _rep in range(reps):
            # ---- per-body SBUF tensors (bufs=1 rotation orders reps)
            wq_s = persist.tile([P, SK_T * GD], _bf16, tag="wq")
            wk_s = persist.tile([P, SK_T * GD], _bf16, tag="wk")
            wv_s = persist.tile([P, SK_T * GD], _bf16, tag="wv")
            wo_s = persist.tile([P, GH * S], _bf16, tag="wo")
            cs_s = persist.tile([P, S], _bf16, tag="cs")
            ss_s = persist.tile([P, S], _bf16, tag="ss")
            md_s = persist.tile([P, SK_T * P], _bf16, tag="md")
            ones_s = persist.tile([P, P], _bf16, tag="ones")
            kt = kvp.tile([P, GH * S], _bf16, tag="kt")    # K^T per head
            vn = kvp.tile([P, SK_T * GD], _bf16, tag="vn")  # V natural

            # ---- initial loads, spread across engine queues
            # SP: wq tiles (needed first), then small constants
            nc.sync.dma_start(ones_s[:], ones[:])
            nc.sync.dma_start(cs_s[:], cs[:])
            nc.sync.dma_start(ss_s[:], ss[:])
            for t in range(SK_T):
                nc.sync.dma_start(wq_s[:, t * GD:(t + 1) * GD], wq[t * P:(t + 1) * P, :])
            # ACT queue: wk, wv then wo (needed later)
            for t in range(SK_T):
                nc.scalar.dma_start(wk_s[:, t * GD:(t + 1) * GD], wk[t * P:(t + 1) * P, :])
                nc.scalar.dma_start(wv_s[:, t * GD:(t + 1) * GD], wv[t * P:(t + 1) * P, :])
            for dt in range(GH):
                nc.scalar.dma_start(wo_s[:, dt * S:(dt + 1) * S], wo[dt * P:(dt + 1) * P, :])

            def _load_xu(u):
                xu = xp.tile([P, SK_T * 512], _bf16, tag="xu")
                for t in range(SK_T):
                    nc.gpsimd.dma_start(
                        xu[:, t * 512:(t + 1) * 512],
                        xT[t * P:(t + 1) * P, u * 512:(u + 1) * 512])
                return xu

            xus = [None] * SQ_U
            xus[0] = _load_xu(0)
            # mask triangle blocks (Pool queue, needed from A(0))
            nc.gpsimd.dma_start(md_s[:], maskd[:])

            aots = [None] * SQ_U

            def _phase_p(u):
                if xus[u] is None:
                    xus[u] = _load_xu(u)
                xu = xus[u]
                qt_u = qtp.tile([P, GH * 512], _bf16, tag="qt")
                # Q^T and K^T (transposed) with fused RoPE
                for dt in range(GH):
                    for (w_s, dst, doff) in (
                            (wq_s, qt_u, dt * 512),
                            (wk_s, kt, dt * S + u * 512)):
                        pq = ps_q.tile([P, 512], _f32, tag="pq")
                        for t in range(SK_T):
                            nc.tensor.matmul(
                                pq[:],
                                w_s[:, t * GD + dt * P: t * GD + dt * P + P],
                                xu[:, t * 512:(t + 1) * 512],
                                start=(t == 0), stop=(t == SK_T - 1))
                        su = slice(u * 512, (u + 1) * 512)
                        t1 = ropep.tile([P, 512], _bf16, tag="t1")
                        t2 = ropep.tile([P, 512], _bf16, tag="t2")
                        nc.vector.tensor_mul(t1[:], pq[:], cs_s[:, su])
                        nc.vector.tensor_mul(t2[0:64, :], pq[64:P, :], ss_s[0:64, su])
                        nc.vector.tensor_mul(t2[64:P, :], pq[0:64, :], ss_s[64:P, su])
                        nc.vector.tensor_add(dst[:, doff:doff + 512], t1[:], t2[:])
                # V (natural layout)
                for st in range(4):
                    g = 4 * u + st
                    pv = ps_v.tile([P, GD], _f32, tag="pv")
                    for t in range(SK_T):
                        nc.tensor.matmul(
                            pv[:],
                            xu[:, t * 512 + st * P: t * 512 + st * P + P],
                            wv_s[:, t * GD:(t + 1) * GD],
                            start=(t == 0), stop=(t == SK_T - 1))
                    nc.scalar.copy(vn[:, g * GD:(g + 1) * GD], pv[:])
                return qt_u

            def _w_group(u, ot):
                # one output-projection group: po2 = sum_dt wo[dt]^T aot[dt]
                aot = aots[u]
                po2 = ps_v.tile([P, 512], _f32, tag="pv")
                for dt in range(GH):
                    nc.tensor.matmul(
                        po2[:],
                        wo_s[:, dt * S + ot * P: dt * S + (ot + 1) * P],
                        aot[:, dt * 512:(dt + 1) * 512],
                        start=(dt == 0), stop=(dt == GH - 1))
                so = sop.tile([P, 512], _bf16, tag="so")
                nc.scalar.copy(so[:], po2[:])
                nc.gpsimd.dma_start(
                    outT[ot * P:(ot + 1) * P, u * 512:(u + 1) * 512], so[:])

            def _phase_a(u, qt_u, w_u):
                # prefetch next x slice
                if u + 1 < SQ_U:
                    xus[u + 1] = _load_xu(u + 1)
                aot = aotp.tile([P, GH * 512], _bf16, tag="aot")
                n_sk = 4 * (u + 1)
                w_pend = list(range(SK_T)) if w_u is not None else []
                slot = [0]

                def _w_tick():
                    # interleave one pending W(u-1) group every w_stride slots
                    slot[0] += 1
                    if w_pend and slot[0] % w_stride == 0:
                        _w_group(w_u, w_pend.pop(0))

                for h in range(GH):
                    qu = qt_u[:, h * 512:(h + 1) * 512]
                    psa = ps_a.tile([P, 512], _f32, tag="a")
                    dacc = denp.tile([P, 512], _bf16, tag="dacc")
                    pts = [None] * n_sk
                    rngs = [None] * n_sk

                    def _consume(t):
                        rng = rngs[t]
                        nc.tensor.matmul(psa[:, rng], vn[:, t * GD + h * P: t * GD + (h + 1) * P],
                                         pts[t][:, rng],
                                         start=(t == 0), stop=(t == n_sk - 1))
                        if t == 0:
                            nc.vector.tensor_scalar_add(dacc[:], pts[0][:], 0.0)
                        else:
                            nc.vector.tensor_add(
                                dacc[:, rng], dacc[:, rng], pts[t][:, rng])

                    for t in range(n_sk):
                        # causal trim: diagonal tiles only need q >= (t%4)*P
                        s0 = 0 if t < 4 * u else (t - 4 * u) * P
                        rng = slice(s0, 512)
                        rngs[t] = rng
                        pss = ps_s.tile([P, 512], _f32, tag="s")
                        nc.tensor.matmul(pss[:, rng],
                                         kt[:, h * S + t * P: h * S + (t + 1) * P],
                                         qu[:, rng], start=True, stop=True)
                        if t >= 4 * u:
                            # mask only the leading [P, P] triangle block
                            nc.vector.tensor_add(
                                pss[:, s0:s0 + P], pss[:, s0:s0 + P],
                                md_s[:, t * P:(t + 1) * P])
                        pt = app.tile([P, 512], _bf16, tag="p")
                        nc.scalar.activation(pt[:, rng], pss[:, rng],
                                             mybir.ActivationFunctionType.Exp)
                        pts[t] = pt
                        if t >= depth:
                            _consume(t - depth)
                        _w_tick()
                    for t in range(max(0, n_sk - depth), n_sk):
                        _consume(t)
                        _w_tick()
                    psd = ps_q.tile([P, 512], _f32, tag="pq")
                    nc.tensor.matmul(psd[:], ones_s[:], dacc[:],
                                     start=True, stop=True)
                    rec = recp.tile([P, 512], _f32, tag="rec")
                    nc.vector.reciprocal(rec[:], psd[:])
                    nc.vector.tensor_mul(
                        aot[:, h * 512:(h + 1) * 512], psa[:], rec[:])
                # drain any leftover W groups
                for ot in w_pend:
                    _w_group(w_u, ot)
                return aot

            # order: P0 A0 P1 [W0|A1] P2 [W1|A2] P3 [W2|A3] W3
            do_a, do_w = "a" in phases, "w" in phases
            qt0 = _phase_p(0)
            if do_a:
                aots[0] = _phase_a(0, qt0, None)
            for u in range(1, SQ_U):
                qt_u = _phase_p(u)
                if do_a:
                    aots[u] = _phase_a(u, qt_u, u - 1 if do_w else None)
            if do_w:
                for ot in range(SK_T):
                    _w_group(SQ_U - 1, ot)
    nc.compile()
    return nc


def _build_generic(reps: int = 1, phases: str = "paw",
                   xu_bufs: int = 3, s_bufs: int = 3, o_bufs: int = 1, depth: int = 3,
                   app_bufs: int = 6, aq_bufs: int = 2, pt_bufs: int = 3,
                   pq_bufs: int = 5, pv_bufs: int = 3, po_bufs: int = 4):
    """v1 path for arbitrary (non-causal) masks; f32r, DRAM staging."""
    nc = bacc.Bacc("TRN2", target_bir_lowering=False, debug=False)
    xT = nc.dram_tensor("xT", [D, S], _f32r, kind="ExternalInput").ap()
    wq = nc.dram_tensor("wq", [D, GD], _f32r, kind="ExternalInput").ap()
    wk = nc.dram_tensor("wk", [D, GD], _f32r, kind="ExternalInput").ap()
    wv = nc.dram_tensor("wv", [D, GD], _f32r, kind="ExternalInput").ap()
    wo = nc.dram_tensor("wo", [GD, D], _f32r, kind="ExternalInput").ap()
    cs = nc.dram_tensor("cs", [P, S], _f32, kind="ExternalInput").ap()
    ss = nc.dram_tensor("ss", [P, S], _f32, kind="ExternalInput").ap()
    ones = nc.dram_tensor("ones", [P, P], _f32r, kind="ExternalInput").ap()
    maskf = nc.dram_tensor("maskf", [S, S], _f32, kind="ExternalInput").ap()
    outT = nc.dram_tensor("outT", [D, S], _f32, kind="ExternalOutput").ap()

    with tile.TileContext(nc) as tc:
      for _rep in range(reps):
        with (
            tc.tile_pool(name="persist", bufs=1) as persist,
            tc.tile_pool(name="dram", bufs=1, space="DRAM") as dpool,
        ):
            qtd = dpool.tile([P, GH * S], _f32r, tag="qtd")   # Q^T per head [hd, S]
            ktd = dpool.tile([P, GH * S], _f32r, tag="ktd")
            vd = dpool.tile([P, SK_T * GD], _f32r, tag="vd")  # V natural
            ones_s = persist.tile([P, P], _f32r, tag="ones")
            nc.sync.dma_start(ones_s[:], ones[:])

            # ---- Phase P: Q^T/K^T/V projections + RoPE -> DRAM scratch
            with (
                tc.tile_pool(name="pw", bufs=1) as pw,
                tc.tile_pool(name="px", bufs=xu_bufs) as px,
                tc.tile_pool(name="pt", bufs=pt_bufs) as ptp,
                tc.tile_pool(name="po", bufs=po_bufs) as po,
                tc.tile_pool(name="ps1", bufs=pq_bufs, space="PSUM") as ps1,
                tc.tile_pool(name="ps2", bufs=pv_bufs, space="PSUM") as ps2,
            ):
                wq_s = pw.tile([P, SK_T * GD], _f32r, tag="wq")
                wk_s = pw.tile([P, SK_T * GD], _f32r, tag="wk")
                wv_s = pw.tile([P, SK_T * GD], _f32r, tag="wv")
                cs_s = pw.tile([P, S], _f32, tag="cs")
                ss_s = pw.tile([P, S], _f32, tag="ss")

                def _load_xu(u):
                    xh = []
                    for half in range(2):
                        xu = px.tile([P, 8 * 512], _f32r, tag="xu")
                        for j in range(8):
                            t = half * 8 + j
                            nc.sync.dma_start(
                                xu[:, j * 512:(j + 1) * 512],
                                xT[t * P:(t + 1) * P, u * 512:(u + 1) * 512])
                        xh.append(xu)
                    return xh

                nc.sync.dma_start(cs_s[:], cs[:])
                nc.sync.dma_start(ss_s[:], ss[:])
                xh0 = _load_xu(0)
                for t in range(SK_T):
                    nc.sync.dma_start(wq_s[:, t * GD:(t + 1) * GD], wq[t * P:(t + 1) * P, :])
                for t in range(SK_T):
                    nc.sync.dma_start(wk_s[:, t * GD:(t + 1) * GD], wk[t * P:(t + 1) * P, :])
                    nc.sync.dma_start(wv_s[:, t * GD:(t + 1) * GD], wv[t * P:(t + 1) * P, :])
                for u in range(SQ_U):
                    xh = xh0 if u == 0 else _load_xu(u)
                    su = slice(u * 512, (u + 1) * 512)
                    for (w_s, dst) in ((wq_s, qtd), (wk_s, ktd)):
                        for dt in range(GH):
                            pq = ps1.tile([P, 512], _f32, tag="pq")
                            for t in range(SK_T):
                                nc.tensor.matmul(
                                    pq[:],
                                    w_s[:, t * GD + dt * P: t * GD + dt * P + P],
                                    xh[t // 8][:, (t % 8) * 512:(t % 8 + 1) * 512],
                                    start=(t == 0), stop=(t == SK_T - 1))
                            t1 = ptp.tile([P, 512], _f32, tag="t1")
                            t2 = ptp.tile([P, 512], _f32, tag="t2")
                            nc.vector.tensor_mul(t1[:], pq[:], cs_s[:, su])
                            nc.vector.tensor_mul(t2[0:64, :], pq[64:P, :], ss_s[0:64, su])
                            nc.vector.tensor_mul(t2[64:P, :], pq[0:64, :], ss_s[64:P, su])
                            ro = po.tile([P, 512], _f32r, tag="ro")
                            nc.vector.tensor_add(ro[:], t1[:], t2[:])
                            nc.sync.dma_start(
                                dst[:, dt * S + u * 512: dt * S + (u + 1) * 512], ro[:])
                    for st in range(4):
                        g = 4 * u + st
                        pv = ps2.tile([P, GD], _f32, tag="pv")
                        for t in range(SK_T):
                            nc.tensor.matmul(
                                pv[:],
                                xh[t // 8][:, (t % 8) * 512 + st * P: (t % 8) * 512 + st * P + P],
                                wv_s[:, t * GD:(t + 1) * GD],
                                start=(t == 0), stop=(t == SK_T - 1))
                        vo = po.tile([P, GD], _f32r, tag="vo")
                        nc.scalar.copy(vo[:], pv[:])
                        nc.sync.dma_start(vd[:, g * GD:(g + 1) * GD], vo[:])

            # ---- Phases A (attention) + W (output projection)
            with (
                tc.tile_pool(name="amask", bufs=1) as amask_p,
                tc.tile_pool(name="akv", bufs=1) as akv,
                tc.tile_pool(name="aq", bufs=aq_bufs) as aq,
                tc.tile_pool(name="app", bufs=app_bufs) as app,
                tc.tile_pool(name="ar", bufs=2) as ar,
                tc.tile_pool(name="aw", bufs=1) as aw,
                tc.tile_pool(name="wst", bufs=3) as wst,
                tc.tile_pool(name="ps3", bufs=s_bufs, space="PSUM") as ps3,
                tc.tile_pool(name="ps3b", bufs=2, space="PSUM") as ps3b,
                tc.tile_pool(name="ps4", bufs=o_bufs, space="PSUM") as ps4,
            ):
                wo_s = aw.tile([P, GH * S], _f32r, tag="wo")
                aot = aw.tile([P, GH * S], _f32r, tag="aot")
                for dt in range(GH):
                    nc.sync.dma_start(wo_s[:, dt * S:(dt + 1) * S], wo[dt * P:(dt + 1) * P, :])
                kt_all = akv.tile([P, GH * S], _f32r, tag="kt")
                v_all = akv.tile([P, GH * S], _f32r, tag="vh")
                for u in range(SQ_U):
                    for lu in (range(SQ_U) if u == 0 else []):
                        for h in range(GH):
                            nc.sync.dma_start(
                                kt_all[:, h * S + lu * 512: h * S + (lu + 1) * 512],
                                ktd[:, h * S + lu * 512: h * S + (lu + 1) * 512])
                            for st in range(4):
                                t = 4 * lu + st
                                nc.sync.dma_start(
                                    v_all[:, h * S + t * P: h * S + (t + 1) * P],
                                    vd[:, t * GD + h * P: t * GD + (h + 1) * P])
                    for h in range(GH):
                        kt_h = kt_all[:, h * S: (h + 1) * S]
                        v_h = v_all[:, h * S: (h + 1) * S]
                        qu = aq.tile([P, 512], _f32r, tag="qu")
                        nc.sync.dma_start(
                            qu[:], qtd[:, h * S + u * 512: h * S + (u + 1) * 512])
                        mu = amask_p.tile([P, SK_T * 512], _f32, tag="mu")
                        for t in range(SK_T):
                            nc.sync.dma_start(
                                mu[:, t * 512:(t + 1) * 512],
                                maskf[t * P:(t + 1) * P, u * 512:(u + 1) * 512])
                        n_sk = SK_T
                        psd = ps3b.tile([P, 512], _f32, tag="d")
                        psa = ps3b.tile([P, 512], _f32, tag="a")
                        pts = [None] * n_sk

                        def _consume(t):
                            nc.tensor.matmul(psd[:], ones_s[:], pts[t][:],
                                             start=(t == 0), stop=(t == n_sk - 1))
                            nc.tensor.matmul(psa[:], v_h[:, t * P:(t + 1) * P], pts[t][:],
                                             start=(t == 0), stop=(t == n_sk - 1))

                        for t in range(n_sk):
                            pss = ps3.tile([P, 512], _f32, tag="s")
                            nc.tensor.matmul(pss[:], kt_h[:, t * P:(t + 1) * P], qu[:],
                                             start=True, stop=True)
                            nc.vector.tensor_add(
                                pss[:], pss[:], mu[:, t * 512:(t + 1) * 512])
                            pt = app.tile([P, 512], _f32r, tag="p")
                            nc.scalar.activation(pt[:], pss[:],
                                                 mybir.ActivationFunctionType.Exp)
                            pts[t] = pt
                            if t >= depth:
                                _consume(t - depth)
                        for t in range(max(0, n_sk - depth), n_sk):
                            _consume(t)
                        rec = ar.tile([P, 512], _f32, tag="rec")
                        nc.vector.reciprocal(rec[:], psd[:])
                        nc.vector.tensor_mul(
                            aot[:, h * S + u * 512: h * S + (u + 1) * 512],
                            psa[:], rec[:])
                    for ot in range(SK_T):
                        po2 = ps4.tile([P, 512], _f32, tag="o")
                        for dt in range(GH):
                            nc.tensor.matmul(
                                po2[:],
                                wo_s[:, dt * S + ot * P: dt * S + (ot + 1) * P],
                                aot[:, dt * S + u * 512: dt * S + (u + 1) * 512],
                                start=(dt == 0), stop=(dt == GH - 1))
                        so = wst.tile([P, 512], _f32, tag="so")
                        nc.scalar.copy(so[:], po2[:])
                        nc.sync.dma_start(
                            outT[ot * P:(ot + 1) * P, u * 512:(u + 1) * 512], so[:])
    nc.compile()
    return nc


class _Runner:
    """Persistent PJRT executable for one compiled Bass module (SPMD over 8 cores)."""

    def __init__(self, nc, n_cores):
        import jax
        from jax.sharding import Mesh, PartitionSpec
        from jax.experimental.shard_map import shard_map
        from concourse.bass2jax import (
            _bass_exec_p, install_neuronx_cc_hook, partition_id_tensor)

        install_neuronx_cc_hook()
        self.jax = jax
        self.n_cores = n_cores
        partition_name = nc.partition_id_tensor.name if nc.partition_id_tensor else None
        in_names, out_names, out_avals = [], [], []
        for alloc in nc.m.functions[0].allocations:
            if not isinstance(alloc, mybir.MemoryLocationSet):
                continue
            name = alloc.memorylocations[0].name
            if alloc.kind == "ExternalInput":
                if name != partition_name:
                    in_names.append(name)
            elif alloc.kind == "ExternalOutput":
                out_names.append(name)
                out_avals.append(jax.core.ShapedArray(
                    tuple(alloc.tensor_shape), mybir.dt.np(alloc.dtype)))
        self.in_names, self.out_names, self.out_avals = in_names, out_names, out_avals
        n_params, n_outs = len(in_names), len(out_avals)
        all_in = list(in_names) + list(out_names)
        if partition_name is not None:
            all_in.append(partition_name)

        def _body(*args):
            operands = list(args)
            if partition_name is not None:
                operands.append(partition_id_tensor())
            return tuple(_bass_exec_p.bind(
                *operands,
                out_avals=tuple(out_avals), in_names=tuple(all_in),
                out_names=tuple(out_names), lowering_input_output_aliases=(),
                sim_require_finite=True, sim_require_nnan=True, nc=nc))

        devices = jax.devices()[:n_cores]
        mesh = Mesh(np.asarray(devices), ("core",))
        self.sharding = jax.sharding.NamedSharding(mesh, PartitionSpec("core"))
        self.fn = jax.jit(
            shard_map(_body, mesh=mesh,
                      in_specs=(PartitionSpec("core"),) * (n_params + n_outs),
                      out_specs=(PartitionSpec("core"),) * n_outs,
                      check_rep=False),
            keep_unused=True)
        self._dev_args = None

    def put_inputs(self, in_maps):
        jax = self.jax
        concat_in = [
            np.concatenate([np.asarray(in_maps[c][n]) for c in range(self.n_cores)], axis=0)
            for n in self.in_names]
        concat_zeros = [
            np.zeros((self.n_cores * a.shape[0], *a.shape[1:]), a.dtype)
            for a in self.out_avals]
        self._dev_args = [
            jax.device_put(v, self.sharding) for v in concat_in + concat_zeros]
        for a in self._dev_args:
            a.block_until_ready()

    def execute(self):
        return self.fn(*self._dev_args)

    def run(self, in_maps):
        last_err = None
        for attempt in range(3):
            try:
                self.put_inputs(in_maps)
                outs = self.execute()
                self.jax.block_until_ready(outs)
                return [
                    {n: np.asarray(outs[i]).reshape(
                        self.n_cores, *self.out_avals[i].shape)[c]
                     for i, n in enumerate(self.out_names)}
                    for c in range(self.n_cores)]
            except Exception as e:  # transient NRT faults: retry
                last_err = e
                import time
                time.sleep(2.0 * (attempt + 1))
        raise last_err


def _get_runner(causal: bool):
    if causal not in _cache:
        _cache[causal] = _Runner(_build(causal), NCORES)
    return _cache[causal]


def _host_prep(x, mask, Wq, Wk, Wv, Wo, causal):
    scale = np.float32(1.0) / np.sqrt(np.float32(HD))
    perm = np.concatenate(
        [np.concatenate([np.arange(0, HD, 2), np.arange(1, HD, 2)]) + HD * hh
         for hh in range(GH)])
    inv = (np.float32(1.0) / np.power(
        np.float32(10000.0),
        np.arange(0, HD, 2).astype(np.float32) / np.float32(HD))).astype(np.float32)
    ang = np.arange(S, dtype=np.float32)[:, None] * inv[None, :]
    cos_t = np.cos(ang).T.astype(np.float32)
    sin_t = np.sin(ang).T.astype(np.float32)
    cs_host = np.ascontiguousarray(np.concatenate([cos_t, cos_t], axis=0))
    ss_host = np.ascontiguousarray(np.concatenate([-sin_t, sin_t], axis=0))
    maskT = np.ascontiguousarray(mask.T)
    if causal:
        dt_w = _np_bf16
        # per key-tile triangle block: mask[q, k].T sliced at the diagonal
        md = np.empty((P, SK_T * P), np.float32)
        for t in range(SK_T):
            q0 = (t // 4) * 512 + (t % 4) * P
            md[:, t * P:(t + 1) * P] = maskT[t * P:(t + 1) * P, q0:q0 + P]
    else:
        dt_w = np.float32
    ones_host = np.ones((P, P), dt_w)
    xTs = [np.ascontiguousarray(x[b].T).astype(dt_w) for b in range(B)]
    in_maps = []
    for c in range(NCORES):
        b, g = c // G, c % G
        rows = slice(g * GD, (g + 1) * GD)
        m = {
            "xT": xTs[b],
            "wq": np.ascontiguousarray(Wq[rows].T[:, perm] * scale).astype(dt_w),
            "wk": np.ascontiguousarray(Wk[rows].T[:, perm]).astype(dt_w),
            "wv": np.ascontiguousarray(Wv[rows].T).astype(dt_w),
            "wo": np.ascontiguousarray(Wo[:, rows].T).astype(dt_w),
            "cs": cs_host.astype(dt_w) if causal else cs_host,
            "ss": ss_host.astype(dt_w) if causal else ss_host,
            "ones": ones_host,
        }
        if causal:
            m["maskd"] = md.astype(_np_bf16)
        else:
            m["maskf"] = maskT
        in_maps.append(m)
    return in_maps


def kernel(x, mask, Wq, Wk, Wv, Wo):
    x = np.asarray(x, dtype=np.float32)
    mask = np.asarray(mask, dtype=np.float32)
    Wq = np.asarray(Wq, dtype=np.float32)
    Wk = np.asarray(Wk, dtype=np.float32)
    Wv = np.asarray(Wv, dtype=np.float32)
    Wo = np.asarray(Wo, dtype=np.float32)
    expected_mask = np.triu(np.full((S, S), -1e9, dtype=np.float32), k=1)
    causal = bool(np.array_equal(mask, expected_mask))
    runner = _get_runner(causal)
    in_maps = _host_prep(x, mask, Wq, Wk, Wv, Wo, causal)
    results = runner.run(in_maps)
    out = np.empty((B, S, D), np.float32)
    for b in range(B):
        acc = results[b * G]["outT"].astype(np.float32)
        for g in range(1, G):
            acc += results[b * G + g]["outT"].astype(np.float32)
        out[b] = acc.T
    return out
